# revision 9
# baseline (speedup 1.0000x reference)
"""ComplexAttention (RoPE'd complex QK, causal softmax, Wo) on 8 Trainium2 cores.

Sharding: cores 0-3 handle batch 0, cores 4-7 batch 1; within a batch group
each core owns 4 heads (tensor-parallel). Per core:
  - project x.T into per-head stacked [re;im] Q/K ([d,t] layout) and V;
    RoPE rotation on eviction: DVE forms the cos/sin products, the PE combines
    halves via constant +/-identity matmuls (avoids cross-partition DVE ops),
  - causal attention per head with scores computed transposed (S^T[k,q]) so
    exp(S^T) is directly the AV-matmul operand; a ones column appended to V
    yields softmax row-sums for free; all matmuls in float32r. Causal masking
    zeroes the fully-acausal prefix via memset and applies a single 128-wide
    triangular mask on the diagonal block,
  - partial output projection with the core's Wo rows, then a 4-chunk
    ReduceScatter (add) over the batch group, overlapped with the Wo matmuls.
    Chunk i of core 4g+r returns rows [512i+128r : 512i+128(r+1)] of batch g.
Host side only shards/stacks inputs and concatenates the 8x4 output chunks.
"""

import numpy as np
import concourse.bacc as bacc
import concourse.tile as tile
from concourse import mybir
from concourse.bass_utils import run_bass_kernel_spmd

F32 = mybir.dt.float32
F32R = mybir.dt.float32r
EXP = mybir.ActivationFunctionType.Exp

B, T, DM, H, D = 2, 2048, 1024, 16, 64
SCALE = D ** -0.5
NCORES = 8
GROUP = 4            # cores per batch
HPC = H // GROUP     # heads per core = 4
TS = 512             # token slice (matmul free dim)
NTS = T // TS        # 4
NDT = DM // 128      # 8 contraction tiles for projections
NKT = T // 128       # 16 key tiles per sequence
VCOL = 65            # D + ones column
ROWS_OUT = T // GROUP  # 512 output rows per core


def _build(collective=True):
    nc = bacc.Bacc("TRN2", debug=False, num_devices=NCORES)

    xt = nc.dram_tensor("xt", [DM, T], F32R, kind="ExternalInput")
    wq = nc.dram_tensor("wq", [DM, HPC * 2 * D], F32R, kind="ExternalInput")
    wk = nc.dram_tensor("wk", [DM, HPC * 2 * D], F32R, kind="ExternalInput")
    wv = nc.dram_tensor("wv", [DM, HPC * D], F32R, kind="ExternalInput")
    wo = nc.dram_tensor("wo", [HPC * D, DM], F32R, kind="ExternalInput")
    csn = nc.dram_tensor("csn", [128, T], F32, kind="ExternalInput")  # [cosT; sinT]
    snc = nc.dram_tensor("snc", [128, T], F32, kind="ExternalInput")  # [sinT; cosT]
    mskd = nc.dram_tensor("mskd", [128, 128], F32, kind="ExternalInput")
    ident = nc.dram_tensor("ident", [128, 128], F32, kind="ExternalInput")
    ones64 = nc.dram_tensor("ones64", [1, 64], F32R, kind="ExternalInput")
    pdif = nc.dram_tensor("pdif", [128, 64], F32R, kind="ExternalInput")  # [I; -I]
    psm = nc.dram_tensor("psm", [128, 64], F32R, kind="ExternalInput")    # [I; I]
    out = nc.dram_tensor("out", [ROWS_OUT, DM], F32, kind="ExternalOutput")

    with tile.TileContext(nc) as tc, \
         nc.allow_low_precision(reason="f32r attention pipeline"):
        with tc.tile_pool(name="persist", bufs=1) as persist, \
             tc.tile_pool(name="wpool", bufs=1) as wpool, \
             tc.tile_pool(name="xtp", bufs=8) as xtp, \
             tc.tile_pool(name="ptp", bufs=4) as ptp, \
             tc.tile_pool(name="tmp", bufs=2) as tmp, \
             tc.tile_pool(name="outp", bufs=2) as outp, \
             tc.tile_pool(name="ps_proj", bufs=2, space="PSUM") as ps_proj, \
             tc.tile_pool(name="ps_tp", bufs=1, space="PSUM") as ps_tp, \
             tc.tile_pool(name="ps_st", bufs=2, space="PSUM") as ps_st, \
             tc.tile_pool(name="ps_av", bufs=1, space="PSUM") as ps_av, \
             tc.tile_pool(name="ps_mix", bufs=2, space="PSUM") as ps_mix, \
             tc.tile_pool(name="dram", bufs=1, space="DRAM") as dram:

            # ---- constants / persistent tensors ----
            csn_t = persist.tile([128, T], F32, tag="csn")
            snc_t = persist.tile([128, T], F32, tag="snc")
            mskd_t = persist.tile([128, 128], F32, tag="mskd")
            id_t = persist.tile([128, 128], F32, tag="ident")
            ones64_t = persist.tile([1, 64], F32R, tag="ones64")
            pdif_t = persist.tile([128, 64], F32R, tag="pdif")
            psm_t = persist.tile([128, 64], F32R, tag="psm")
            nc.sync.dma_start(out=csn_t[:], in_=csn[:])
            nc.sync.dma_start(out=snc_t[:], in_=snc[:])
            nc.sync.dma_start(out=mskd_t[:], in_=mskd[:])
            nc.sync.dma_start(out=id_t[:], in_=ident[:])
            nc.sync.dma_start(out=ones64_t[:], in_=ones64[:])
            nc.sync.dma_start(out=pdif_t[:], in_=pdif[:])
            nc.sync.dma_start(out=psm_t[:], in_=psm[:])

            wo_t = persist.tile([128, 2, DM], F32R, tag="wo")
            nc.sync.dma_start(
                out=wo_t[:], in_=wo.rearrange("(jt p) m -> p jt m", p=128))

            ones16 = persist.tile([128, NKT, 1], F32, tag="ones16")
            nc.vector.memset(ones16[:], 1.0)

            qc = [persist.tile([128, T], F32R, tag=f"qc{h}", name=f"qc{h}")
                  for h in range(HPC)]
            kc = [persist.tile([128, T], F32R, tag=f"kc{h}", name=f"kc{h}")
                  for h in range(HPC)]
            vpair = [persist.tile([128, NKT, 130], F32R, tag=f"v{p}", name=f"v{p}")
                     for p in range(2)]
            attnT = [persist.tile([128, T], F32R, tag=f"at{p}", name=f"at{p}")
                     for p in range(2)]

            for p in range(2):
                nc.vector.tensor_copy(out=vpair[p][:, :, 64:65], in_=ones16[:])
                nc.vector.tensor_copy(out=vpair[p][:, :, 129:130], in_=ones16[:])

            dram_partial = dram.tile([T, DM], F32)
            dram_rs = [dram.tile([ROWS_OUT // GROUP, DM], F32, name=f"rs{i}")
                       for i in range(GROUP)]

            def rope_evict(ps, dst, tcols, alt):
                # ps = [re(0:64); im(64:128)] x TS in PSUM.
                # dst[0:64]  = cos*re - sin*im ; dst[64:128] = sin*re + cos*im.
                # DVE forms mc = ps*[c;s], ms = ps*[s;c]; PE combines halves
                # with [I;-I] / [I;I] matmuls; ACT/DVE alternate evictions.
                mc = tmp.tile([128, TS], F32R, tag="mc")
                ms_ = tmp.tile([128, TS], F32R, tag="ms")
                nc.vector.tensor_mul(mc[:], ps[:], csn_t[:, tcols])
                nc.vector.tensor_mul(ms_[:], ps[:], snc_t[:, tcols])
                pre = ps_mix.tile([64, TS], F32, tag="mix")
                nc.tensor.matmul(pre[:], pdif_t[:], mc[:], start=True, stop=True)
                pim = ps_mix.tile([64, TS], F32, tag="mix")
                nc.tensor.matmul(pim[:], psm_t[:], ms_[:], start=True, stop=True)
                if alt:
                    nc.scalar.copy(out=dst[0:64, :], in_=pre[:])
                    nc.vector.tensor_copy(out=dst[64:128, :], in_=pim[:])
                else:
                    nc.vector.tensor_copy(out=dst[0:64, :], in_=pre[:])
                    nc.scalar.copy(out=dst[64:128, :], in_=pim[:])

            def stage_A(s):
                h0 = 2 * s
                wq_s = wpool.tile([128, NDT, 256], F32R, tag="wq_s")
                wk_s = wpool.tile([128, NDT, 256], F32R, tag="wk_s")
                wv_s = wpool.tile([128, NDT, 128], F32R, tag="wv_s")
                nc.sync.dma_start(
                    out=wq_s[:],
                    in_=wq.rearrange("(dt p) j -> p dt j", p=128)[:, :, s * 256:(s + 1) * 256])
                nc.sync.dma_start(
                    out=wk_s[:],
                    in_=wk.rearrange("(dt p) j -> p dt j", p=128)[:, :, s * 256:(s + 1) * 256])
                nc.sync.dma_start(
                    out=wv_s[:],
                    in_=wv.rearrange("(dt p) j -> p dt j", p=128)[:, :, s * 128:(s + 1) * 128])

                for ts in range(NTS):
                    tcols = slice(ts * TS, (ts + 1) * TS)
                    xts = []
                    for dt_i in range(NDT):
                        xc = xtp.tile([128, TS], F32R, tag="xt")
                        nc.sync.dma_start(
                            out=xc[:],
                            in_=xt[dt_i * 128:(dt_i + 1) * 128, tcols])
                        xts.append(xc)

                    # Q then K for each of the 2 heads, then V (both heads)
                    for hi in range(2):
                        for wi, (w_s, dsts) in enumerate(((wq_s, qc), (wk_s, kc))):
                            p = ps_proj.tile([128, TS], F32, tag="proj")
                            for dt_i in range(NDT):
                                nc.tensor.matmul(
                                    p[:],
                                    w_s[:, dt_i, hi * 128:(hi + 1) * 128],
                                    xts[dt_i][:],
                                    start=(dt_i == 0), stop=(dt_i == NDT - 1))
                            rope_evict(p, dsts[h0 + hi][:, tcols], tcols,
                                       alt=(hi + wi) % 2 == 0)

                    p = ps_proj.tile([128, TS], F32, tag="proj")
                    for dt_i in range(NDT):
                        nc.tensor.matmul(
                            p[:], wv_s[:, dt_i, :], xts[dt_i][:],
                            start=(dt_i == 0), stop=(dt_i == NDT - 1))
                    vt_sb = tmp.tile([128, TS], F32, tag="vt")
                    nc.scalar.copy(out=vt_sb[:], in_=p[:])
                    for sub in range(4):
                        tp = ps_tp.tile([128, 128], F32, tag="tp")
                        nc.tensor.transpose(
                            tp[:], vt_sb[:, sub * 128:(sub + 1) * 128], id_t[:])
                        kt_i = ts * 4 + sub
                        nc.vector.tensor_copy(
                            out=vpair[s][:, kt_i, 0:64], in_=tp[:, 0:64])
                        nc.vector.tensor_copy(
                            out=vpair[s][:, kt_i, 65:129], in_=tp[:, 64:128])

            def stage_B(h):
                pair, half = divmod(h, 2)
                for qs in range(NTS):
                    nk = 4 * (qs + 1)
                    av = ps_av.tile([VCOL, TS], F32, tag="av")
                    for kt_i in range(nk):
                        dpos = kt_i - 4 * qs
                        st = ps_st.tile([128, TS], F32, tag="st")
                        pt = ptp.tile([128, TS], F32R, tag="pt")
                        if dpos < 0:
                            nc.tensor.matmul(
                                st[:],
                                kc[h][:, kt_i * 128:(kt_i + 1) * 128],
                                qc[h][:, qs * TS:(qs + 1) * TS],
                                start=True, stop=True)
                            nc.scalar.activation(
                                out=pt[:], in_=st[:], func=EXP, scale=SCALE)
                        else:
                            c0 = 128 * dpos           # fully-acausal prefix
                            nc.tensor.matmul(
                                st[:, c0:TS],
                                kc[h][:, kt_i * 128:(kt_i + 1) * 128],
                                qc[h][:, qs * TS + c0:(qs + 1) * TS],
                                start=True, stop=True)
                            nc.scalar.activation(
                                out=pt[:, c0:TS], in_=st[:, c0:TS],
                                func=EXP, scale=SCALE)
                            nc.vector.tensor_mul(
                                pt[:, c0:c0 + 128], pt[:, c0:c0 + 128], mskd_t[:])
                        c0 = max(0, 128 * dpos)
                        nc.tensor.matmul(
                            av[:, c0:TS],
                            vpair[pair][:, kt_i, half * 65:(half + 1) * 65],
                            pt[:, c0:TS],
                            start=(kt_i == 0), stop=(kt_i == nk - 1))
                    qcols = slice(qs * TS, (qs + 1) * TS)
                    rrow = tmp.tile([1, TS], F32R, tag="rrow")
                    nc.vector.reciprocal(out=rrow[:], in_=av[64:65, :])
                    bc = ps_mix.tile([64, TS], F32, tag="mix")
                    nc.tensor.matmul(bc[:], ones64_t[:], rrow[:], start=True, stop=True)
                    bcs = tmp.tile([64, TS], F32, tag="bcs")
                    nc.scalar.copy(out=bcs[:], in_=bc[:])
                    nc.vector.tensor_mul(
                        attnT[pair][half * 64:(half + 1) * 64, qcols],
                        av[0:64, :], bcs[:])

            def stage_C():
                for chunk in range(GROUP):
                    for tt in range(4 * chunk, 4 * chunk + 4):
                        trows = slice(tt * 128, (tt + 1) * 128)
                        osb = outp.tile([128, DM], F32, tag="osb")
                        for ms in range(2):
                            p = ps_proj.tile([128, TS], F32, tag="proj")
                            for jt in range(2):
                                nc.tensor.matmul(
                                    p[:],
                                    attnT[jt][:, trows],
                                    wo_t[:, jt, ms * TS:(ms + 1) * TS],
                                    start=(jt == 0), stop=(jt == 1))
                            nc.scalar.copy(out=osb[:, ms * TS:(ms + 1) * TS], in_=p[:])
                        nc.sync.dma_start(out=dram_partial[trows, :], in_=osb[:])
                    if collective:
                        nc.gpsimd.collective_compute(
                            "ReduceScatter",
                            mybir.AluOpType.add,
                            replica_groups=[[0, 1, 2, 3], [4, 5, 6, 7]],
                            ins=[dram_partial[chunk * 512:(chunk + 1) * 512, :].opt()],
                            outs=[dram_rs[chunk].opt()],
                        )
                    else:
                        nc.sync.dma_start(
                            out=dram_rs[chunk][:],
                            in_=dram_partial[chunk * 512:chunk * 512 + 128, :])
                    ob = outp.tile([128, DM], F32, tag="osb")
                    nc.sync.dma_start(out=ob[:], in_=dram_rs[chunk][:])
                    nc.sync.dma_start(out=out[chunk * 128:(chunk + 1) * 128, :], in_=ob[:])

            stage_A(0)
            stage_B(0)
            stage_B(1)
            stage_A(1)
            stage_B(2)
            stage_B(3)
            stage_C()

    nc.compile()
    return nc


_NC_CACHE = []


def _get_nc():
    if not _NC_CACHE:
        _NC_CACHE.append(_build())
    return _NC_CACHE[0]


def _prep_inputs(x, Wq_re, Wq_im, Wk_re, Wk_im, Wv, Wo, cos, sin):
    cosT = np.ascontiguousarray(cos.T).astype(np.float32)   # [64, T]
    sinT = np.ascontiguousarray(sin.T).astype(np.float32)
    csn = np.concatenate([cosT, sinT], axis=0)              # [128, T]
    snc = np.concatenate([sinT, cosT], axis=0)
    kk = np.arange(128)[:, None]
    qq = np.arange(128)[None, :]
    mskd = (kk <= qq).astype(np.float32)
    ident = np.eye(128, dtype=np.float32)
    ones64 = np.ones((1, 64), np.float32)
    eye64 = np.eye(64, dtype=np.float32)
    pdif = np.concatenate([eye64, -eye64], axis=0)          # [128, 64]
    psm = np.concatenate([eye64, eye64], axis=0)

    in_maps = []
    for c in range(NCORES):
        g, r = divmod(c, GROUP)
        heads = [4 * r + i for i in range(HPC)]
        wq_c = np.empty((DM, HPC * 2 * D), np.float32)
        wk_c = np.empty((DM, HPC * 2 * D), np.float32)
        wv_c = np.empty((DM, HPC * D), np.float32)
        wo_c = np.empty((HPC * D, DM), np.float32)
        for i, h in enumerate(heads):
            hs = slice(h * D, (h + 1) * D)
            wq_c[:, 2 * i * D:(2 * i + 1) * D] = Wq_re[:, hs]
            wq_c[:, (2 * i + 1) * D:(2 * i + 2) * D] = Wq_im[:, hs]
            wk_c[:, 2 * i * D:(2 * i + 1) * D] = Wk_re[:, hs]
            wk_c[:, (2 * i + 1) * D:(2 * i + 2) * D] = Wk_im[:, hs]
            wv_c[:, i * D:(i + 1) * D] = Wv[:, hs]
            wo_c[i * D:(i + 1) * D, :] = Wo[hs, :]
        in_maps.append({
            "xt": np.ascontiguousarray(x[g].T).astype(np.float32),
            "wq": wq_c, "wk": wk_c, "wv": wv_c, "wo": wo_c,
            "csn": csn, "snc": snc, "mskd": mskd,
            "ident": ident, "ones64": ones64, "pdif": pdif, "psm": psm,
        })
    return in_maps


def _assemble(outs):
    """outs: list/array of 8 per-core [512, 1024] chunk stacks."""
    full = np.empty((B, T, DM), np.float32)
    for c in range(NCORES):
        g, r = divmod(c, GROUP)
        for i in range(GROUP):
            full[g, 512 * i + 128 * r: 512 * i + 128 * (r + 1), :] = \
                outs[c][i * 128:(i + 1) * 128, :]
    return full


def kernel(**inputs):
    inputs = {k: np.asarray(v) for k, v in inputs.items()}
    nc = _get_nc()
    in_maps = _prep_inputs(**inputs)
    res = run_bass_kernel_spmd(nc, in_maps, core_ids=list(range(NCORES)))
    return _assemble([res.results[c]["out"] for c in range(NCORES)])
